# revision 1
# baseline (speedup 1.0000x reference)
"""KNN palette-retrieval kernel for Trainium2 (8 NeuronCores, data-parallel).

Per pixel of rgb_mask [16,3,512,512]: find the palette row (of 21,
L2-normalized) with max cosine similarity, emit that normalized color;
zero pixels emit 0.  argmax(cos) == argmax(dot) since pixel norm is a
positive scalar, so pixel normalization is skipped.

Layout: each core takes 2 batches = 524288 px, split into 32 "sets" g of
16384 px.  PE row layout = 32*k' + g (quadrant-aligned so every DVE
partition range starts at 0/32/64/96).  24 k-slots = 6 matmuls x 4 slots
(21 real + 3 zero-padded; sims are strictly positive so pads never win).

Per tile of 32x512 pixels:
  mm1_i (x6): pa_i[32k'+g, n] = sims for k=4i+k'        (PE, K=96)
  chain:  s = copy(pa_0) (ACT);  5x running TT-max vs pa_1..pa_5 (DVE)
  fold:   max over quadrants (2x TT, aligned)  -> m[32, n]
  floor:  mfl = max(m, 1e-20)                            (GPSIMD)
  mm2_i (x6): pa_i -= mfl broadcast over k-slots  (PE accumulate, exact
          f32 so the argmax row becomes +0.0)
  oh_i:   is_ge(pa_i, 0) in {0,1}  (DVE tensor_scalar; equality keeps
          the argmax row, zero pixels go all-cold via the floor)
  mm3_i (x6): pout[32c+g] += sum cn[k,c]*oh_i  -> exact palette color
  yout:   copy pout -> SBUF (ACT), DMA out.
"""

import sys

sys.path.insert(0, "/opt/trn_rl_repo")

import numpy as np

B, C, H, W = 16, 3, 512, 512
K = 21
NCORES = 8
BPC = B // NCORES            # batches per core
PXC = BPC * H * W            # pixels per core = 524288
G = 32                       # pixel sets (partition-packed)
REG = PXC // G               # 16384 columns per set
NT = 512                     # pixel columns per tile
NTILES = REG // NT           # 32
NMM = 6                      # k-slot matmuls (6*4 = 24 >= 21)

_CACHE: dict = {}


def _build_nc():
    if "nc" in _CACHE:
        return _CACHE["nc"]
    from contextlib import ExitStack

    import concourse.tile as tile
    from concourse import bacc, mybir

    f32 = mybir.dt.float32
    mx = mybir.AluOpType.max
    ge = mybir.AluOpType.is_ge

    nc = bacc.Bacc("TRN2", target_bir_lowering=False, debug=False,
                   num_devices=NCORES)
    x = nc.dram_tensor("x", [C * G, REG], f32, kind="ExternalInput").ap()
    w1 = nc.dram_tensor("w1", [NMM, C * G, 128], f32,
                        kind="ExternalInput").ap()
    w2 = nc.dram_tensor("w2", [G, 128], f32, kind="ExternalInput").ap()
    w3 = nc.dram_tensor("w3", [NMM, 128, C * G], f32,
                        kind="ExternalInput").ap()
    y = nc.dram_tensor("y", [C * G, REG], f32, kind="ExternalOutput").ap()

    with ExitStack() as ctx:
        tc = ctx.enter_context(tile.TileContext(nc))
        wp = ctx.enter_context(tc.tile_pool(name="w", bufs=1))
        inp = ctx.enter_context(tc.tile_pool(name="xin", bufs=3))
        sp = ctx.enter_context(tc.tile_pool(name="s", bufs=2))
        ohp = ctx.enter_context(tc.tile_pool(name="oh", bufs=2))
        yp = ctx.enter_context(tc.tile_pool(name="y", bufs=3))
        pap = [ctx.enter_context(
            tc.tile_pool(name=f"pa{i}", bufs=1, space="PSUM"))
            for i in range(NMM)]
        pop = ctx.enter_context(tc.tile_pool(name="po", bufs=2, space="PSUM"))

        w1s, w3s = [], []
        for i in range(NMM):
            w1t = wp.tile([C * G, 128], f32, name=f"w1s{i}")
            nc.sync.dma_start(w1t[:], w1[i])
            w1s.append(w1t)
            w3t = wp.tile([128, C * G], f32, name=f"w3s{i}")
            nc.sync.dma_start(w3t[:], w3[i])
            w3s.append(w3t)
        w2s = wp.tile([G, 128], f32)
        nc.sync.dma_start(w2s[:], w2[:])

        for t in range(NTILES):
            c0 = t * NT
            xin = inp.tile([C * G, NT], f32, tag="xin")
            nc.sync.dma_start(xin[:], x[:, c0:c0 + NT])

            pa = []
            for i in range(NMM):
                pai = pap[i].tile([128, NT], f32, tag=f"pa{i}", name=f"pa{i}")
                nc.tensor.matmul(pai[:], w1s[i][:], xin[:],
                                 start=True, stop=False)
                pa.append(pai)

            # running max chain over pa_0..4 (<=1 PSUM operand per TT;
            # SBUF+SBUF inputs must share base partition, PSUM+SBUF is free)
            s = sp.tile([128, NT], f32, tag="s")
            sm = sp.tile([128, NT], f32, tag="sm")
            nc.scalar.copy(s[:], pa[0][:])
            nc.vector.tensor_tensor(sm[:], pa[1][:], s[:], mx)
            nc.vector.tensor_tensor(s[:], pa[2][:], sm[:], mx)
            nc.vector.tensor_tensor(sm[:], pa[3][:], s[:], mx)
            nc.vector.tensor_tensor(s[:], pa[4][:], sm[:], mx)
            # fold: pa_5 (1 real + 3 zero slots) as the PSUM operand lets
            # the 64-row fold cross bases; then equal-base 32-row merges.
            u = sp.tile([64, NT], f32, tag="u")
            nc.vector.tensor_tensor(u[:], pa[5][0:64, :], s[64:128, :], mx)
            m1 = sp.tile([32, NT], f32, tag="m1")
            m2 = sp.tile([32, NT], f32, tag="m2")
            m3 = sp.tile([32, NT], f32, tag="m3")
            nc.vector.tensor_tensor(m1[:], u[0:32, :], s[0:32, :], mx)
            nc.vector.tensor_tensor(m2[:], u[32:64, :], s[32:64, :], mx)
            nc.vector.tensor_tensor(m3[:], m1[:], m2[:], mx)

            mfl = sp.tile([G, NT], f32, tag="mfl")
            nc.gpsimd.tensor_scalar_max(mfl[:], m3[:], 1e-20)

            for i in range(NMM):
                nc.tensor.matmul(pa[i][:], w2s[:], mfl[:],
                                 start=False, stop=True)

            pout = pop.tile([C * G, NT], f32, tag="po")
            for i in range(NMM):
                oh = ohp.tile([128, NT], f32, tag=f"oh{i}", name=f"oh{i}")
                nc.vector.tensor_scalar(oh[:], pa[i][:], 0.0, None, ge)
                nc.tensor.matmul(pout[:], w3s[i][:], oh[:],
                                 start=(i == 0), stop=(i == NMM - 1))

            yout = yp.tile([C * G, NT], f32, tag="yout")
            nc.scalar.copy(yout[:], pout[:])
            nc.sync.dma_start(y[:, c0:c0 + NT], yout[:])

    nc.compile()
    _CACHE["nc"] = nc
    return nc


def _weights(colors: np.ndarray):
    cn = (colors.astype(np.float64)
          / np.linalg.norm(colors.astype(np.float64), axis=-1, keepdims=True))
    W1 = np.zeros((NMM, C * G, 128), np.float32)
    W2 = np.zeros((G, 128), np.float32)
    W3 = np.zeros((NMM, 128, C * G), np.float32)
    for i in range(NMM):
        for kp in range(4):
            k = 4 * i + kp
            if k >= K:
                continue
            for g in range(G):
                for c in range(C):
                    W1[i, G * c + g, G * kp + g] = cn[k, c]
                    W3[i, G * kp + g, G * c + g] = cn[k, c]
    for g in range(G):
        for kp in range(4):
            W2[g, G * kp + g] = -1.0
    return W1, W2, W3


def _stage_inputs(rgb_mask: np.ndarray, colors: np.ndarray):
    W1, W2, W3 = _weights(np.asarray(colors, np.float32))
    in_maps = []
    for i in range(NCORES):
        xc = np.asarray(rgb_mask[BPC * i:BPC * (i + 1)], np.float32)
        xc = np.transpose(xc, (1, 0, 2, 3)).reshape(C * G, REG)
        in_maps.append({
            "x": np.ascontiguousarray(xc),
            "w1": W1, "w2": W2, "w3": W3,
        })
    return in_maps


def _gather_outputs(results):
    outs = []
    for i in range(NCORES):
        yb = results[i]["y"].reshape(C, BPC, H, W)
        outs.append(np.transpose(yb, (1, 0, 2, 3)))
    return np.ascontiguousarray(np.concatenate(outs, axis=0))


def run(rgb_mask, colors, trace=False, **kw):
    from concourse.bass_utils import run_bass_kernel_spmd

    nc = _build_nc()
    in_maps = _stage_inputs(rgb_mask, colors)
    res = run_bass_kernel_spmd(nc, in_maps, core_ids=list(range(NCORES)),
                               trace=trace, **kw)
    return _gather_outputs(res.results), res


def kernel(rgb_mask, colors):
    out, _ = run(rgb_mask, colors)
    return out



# revision 25
# speedup vs baseline: 92.8801x; 92.8801x over previous
"""KNN palette-retrieval kernel for Trainium2 (8 NeuronCores, data-parallel).

Per pixel of rgb_mask [16,3,512,512]: find the palette row (of 21,
L2-normalized) with max cosine similarity, emit that normalized color;
zero pixels emit 0.  argmax(cos) == argmax(dot) since pixel norm is a
positive scalar, so pixel normalization is skipped.

Layout: each core takes 2 batches = 524288 px, split into 32 "sets" g of
16384 px.  PE row layout = 32*k' + g (quadrant-aligned so every DVE
partition range starts at 0/32/64/96).  24 k-slots = 6 matmuls x 4 slots
(21 real + 3 zero-padded; sims are strictly positive so pads never win).

All sims-path matmuls are fp32 (fp32r was probed on this HW: it rounds
inputs to ~tf32, which flips ~1.3% of argmaxes and breaks the exact
winner-residual — unusable at rel-err 2e-2).  mm3 runs in bf16: its
one-hot inputs are exact {-1,0,+1} and the output is the bf16-rounded
palette color.

Per tile of 32x512 pixels:
  mm1_i (x6, f32): pa_i[32k'+g, n] = sims for k=4i+k'   (PE, K=96)
  chain:  s = copy(pa_0) (ACT); 4x running TT-max vs pa_1..pa_4 (DVE)
  fold:   u = max(pa_5[0:64], s[64:128]) (DVE; pa_5's zero-pad slots let
          the 64-row fold cross partition bases), two 32-row folds and
          the floor (max with DF) fused via scalar_tensor_tensor (DVE)
  mm2_i (x6, f32): pa_i -= mfl broadcast over k-slots (PE accumulate,
          exact: the winner row becomes +0.0)
  oh_i:   sign(pa_i + D2) in {-1,0,+1} bf16 on ACT (all six groups;
          group 5 only has 1 real slot so only its [0:32] rows).
          Winner +0.0 -> +1; zero pixels go all -1 and cancel to 0.
  mm3 (x7, bf16): pout += W3_i^T oh_i with halved palette weights, plus
          a constant correction matmul (ones x w3c) adding back exactly
          the f32 sum of the bf16 weights (split hi+lo over two ones
          rows), so for winner w: out = cn_w + 2*bf16err(cn_w/2) and all
          other bf16 rounding errors cancel exactly.
  yout:   copy pout -> y chunk in SBUF (ACT); store per 4 tiles (big DMA
          on the sync ring; x loads on sync, weights on the scalar ring).

PSUM: pa0 double-buffered (2 banks), pa1-4 single (4), pa5 shares the
2-slot pool with pout (lifetimes interleave) -> 8 banks.  The one-hot
order (1,2,5,3,4,0) frees pa1 first (the next tile's first chain TT
reads it); pa0's extra buffer decouples the s-copy from the recycle
loop.

`reps` wraps the whole body in a hardware For_i loop (identical
addresses each iteration) — used by test.py's differential timing.
"""

import sys

sys.path.insert(0, "/opt/trn_rl_repo")

import numpy as np

B, C, H, W = 16, 3, 512, 512
K = 21
NCORES = 8
BPC = B // NCORES            # batches per core
PXC = BPC * H * W            # pixels per core = 524288
G = 32                       # pixel sets (partition-packed)
REG = PXC // G               # 16384 columns per set
NT = 512                     # pixel columns per tile
NTILES = REG // NT           # 32
NMM = 6                      # k-slot matmuls (6*4 = 24 >= 21)
NCH = 8                      # x/y DMA chunks
CHW = REG // NCH             # 2048 columns per chunk
TPC = CHW // NT              # tiles per chunk = 4

DF = 2.0 ** -14              # max floor (zero-pixel handling)
D2 = 2.0 ** -24              # sign bias so the winner's +0.0 maps to +1

_CACHE: dict = {}


def _build_nc(reps=1):
    key = ("nc", reps)
    if key in _CACHE:
        return _CACHE[key]
    from contextlib import ExitStack

    import concourse.tile as tile
    from concourse import bacc, mybir

    f32 = mybir.dt.float32
    bf16 = mybir.dt.bfloat16
    mx = mybir.AluOpType.max

    nc = bacc.Bacc("TRN2", target_bir_lowering=False, debug=False,
                   num_devices=NCORES)
    x = nc.dram_tensor("x", [C * G, REG], f32, kind="ExternalInput").ap()
    w1 = nc.dram_tensor("w1", [NMM, C * G, 128], f32,
                        kind="ExternalInput").ap()
    w2 = nc.dram_tensor("w2", [G, 128], f32, kind="ExternalInput").ap()
    w3 = nc.dram_tensor("w3", [NMM, 128, C * G], bf16,
                        kind="ExternalInput").ap()
    w3c = nc.dram_tensor("w3c", [G, C * G], bf16, kind="ExternalInput").ap()
    ones = nc.dram_tensor("ones", [G, NT], bf16, kind="ExternalInput").ap()
    y = nc.dram_tensor("y", [C * G, REG], f32, kind="ExternalOutput").ap()

    with ExitStack() as ctx:
        tc = ctx.enter_context(tile.TileContext(nc))
        wp = ctx.enter_context(tc.tile_pool(name="w", bufs=1))
        xp = ctx.enter_context(tc.tile_pool(name="xch", bufs=NCH + 2))
        sp = ctx.enter_context(tc.tile_pool(name="s", bufs=2))
        fp = ctx.enter_context(tc.tile_pool(name="f", bufs=3))
        ohp = ctx.enter_context(tc.tile_pool(name="oh", bufs=2))
        yp = ctx.enter_context(tc.tile_pool(name="ych", bufs=3))
        pap = [ctx.enter_context(
            tc.tile_pool(name=f"pa{i}", bufs=(2 if i == 0 else 1),
                         space="PSUM"))
            for i in range(NMM - 1)]
        pop = ctx.enter_context(tc.tile_pool(name="po", bufs=2, space="PSUM"))

        w1s, w3s = [], []
        for i in range(NMM):
            w1t = wp.tile([C * G, 128], f32, name=f"w1s{i}")
            nc.scalar.dma_start(w1t[:], w1[i])
            w1s.append(w1t)
            w3t = wp.tile([128, C * G], bf16, name=f"w3s{i}")
            nc.scalar.dma_start(w3t[:], w3[i])
            w3s.append(w3t)
        w2s = wp.tile([G, 128], f32, name="w2s")
        nc.scalar.dma_start(w2s[:], w2[:])
        w3cs = wp.tile([G, C * G], bf16, name="w3cs")
        nc.scalar.dma_start(w3cs[:], w3c[:])
        oness = wp.tile([G, NT], bf16, name="oness")
        nc.scalar.dma_start(oness[:], ones[:])
        d2t = wp.tile([128, 1], f32, name="d2t")
        nc.gpsimd.memset(d2t[:], D2)

        def body():
            for ch in range(NCH):
                b0 = ch * CHW
                xch = xp.tile([C * G, CHW], f32, tag="xch")
                nc.sync.dma_start(xch[:], x[:, b0:b0 + CHW])
                ych = yp.tile([C * G, CHW], f32, tag="ych")

                for tt in range(TPC):
                    c0 = tt * NT
                    xin = xch[:, c0:c0 + NT]

                    pa = []
                    for i in range(NMM):
                        if i < NMM - 1:
                            pai = pap[i].tile([128, NT], f32, tag=f"pa{i}",
                                              name=f"pa{i}")
                        else:
                            pai = pop.tile([128, NT], f32, tag="po",
                                           name="pa5")
                        nc.tensor.matmul(pai[:], w1s[i][:], xin,
                                         start=True, stop=True)
                        pa.append(pai)

                    # running max chain (<=1 PSUM operand per TT)
                    s = sp.tile([128, NT], f32, tag="s")
                    sm = sp.tile([128, NT], f32, tag="sm")
                    nc.scalar.copy(s[:], pa[0][:])
                    nc.vector.tensor_tensor(sm[:], pa[1][:], s[:], mx)
                    nc.vector.tensor_tensor(s[:], pa[2][:], sm[:], mx)
                    nc.vector.tensor_tensor(sm[:], pa[3][:], s[:], mx)
                    nc.vector.tensor_tensor(s[:], pa[4][:], sm[:], mx)
                    u = fp.tile([64, NT], f32, tag="u")
                    nc.vector.tensor_tensor(u[:], pa[5][0:64, :],
                                            s[64:128, :], mx)
                    m1 = fp.tile([32, NT], f32, tag="m1")
                    m2 = fp.tile([32, NT], f32, tag="m2")
                    mfl = fp.tile([G, NT], f32, tag="mfl")
                    nc.vector.tensor_tensor(m1[:], u[0:32, :], s[0:32, :], mx)
                    nc.vector.tensor_tensor(m2[:], u[32:64, :],
                                            s[32:64, :], mx)
                    nc.vector.scalar_tensor_tensor(mfl[:], m1[:], DF, m2[:],
                                                   mx, mx)

                    for i in range(NMM):
                        nc.tensor.matmul(pa[i][:], w2s[:], mfl[:],
                                         start=False, stop=True,
                                         skip_group_check=True)

                    pout = pop.tile([C * G, NT], f32, tag="po")
                    # constant correction first: it has no data deps
                    nc.tensor.matmul(pout[:], w3cs[:], oness[:],
                                     start=True, stop=False)
                    # order: group 1 first — the next tile's first chain TT
                    # reads pa1, so free its bank earliest.  pa0 is double-
                    # buffered, so its one-hot can go last.
                    for i in (1, 2, 5, 3, 4, 0):
                        rows = 32 if i == NMM - 1 else 128
                        oh = ohp.tile([rows, NT], bf16, tag=f"oh{i}",
                                      name=f"oh{i}")
                        nc.scalar.sign(oh[:], pa[i][0:rows, :],
                                       bias=d2t[0:rows, :])
                        nc.tensor.matmul(pout[:], w3s[i][0:rows, :], oh[:],
                                         start=False, stop=(i == 0))

                    nc.scalar.copy(ych[:, c0:c0 + NT], pout[:])

                nc.sync.dma_start(y[:, b0:b0 + CHW], ych[:])

        if reps == 1:
            body()
        else:
            with tc.For_i(0, reps, 1):
                body()

    nc.compile()
    _CACHE[key] = nc
    return nc


def _weights(colors: np.ndarray):
    from concourse import mybir
    bf16 = mybir.dt.np(mybir.dt.bfloat16)

    cn = (colors.astype(np.float64)
          / np.linalg.norm(colors.astype(np.float64), axis=-1, keepdims=True))
    cn32 = cn.astype(np.float32)
    W1 = np.zeros((NMM, C * G, 128), np.float32)
    W2 = np.zeros((G, 128), np.float32)
    W3 = np.zeros((NMM, 128, C * G), np.float32)
    for i in range(NMM):
        for kp in range(4):
            k = 4 * i + kp
            if k >= K:
                continue
            for g in range(G):
                for c in range(C):
                    W1[i, G * c + g, G * kp + g] = cn32[k, c]
                    W3[i, G * kp + g, G * c + g] = cn32[k, c] * 0.5
    for g in range(G):
        for kp in range(4):
            W2[g, G * kp + g] = -1.0
    # The one-hot is sign-encoded ({-1,0,+1} * cn_k/2), so the ones-row
    # correction must add back EXACTLY the f32 sum of the bf16-rounded
    # halved weights: for winner w the output is then
    #   (cn_w/2+e_w) - sum_{k!=w}(cn_k/2+e_k) + sum_k(cn_k/2+e_k)
    #   = cn_w + 2 e_w   (all other rounding errors cancel exactly).
    # Encode that sum as hi+lo bf16 on two ones-rows (~2^-18 residual).
    W3bf = W3.astype(bf16)
    csum = np.zeros(C, np.float64)
    for i in range(NMM):
        for kp in range(4):
            k = 4 * i + kp
            if k < K:
                for c in range(C):
                    csum[c] += np.float64(
                        W3bf[i, G * kp, G * c].astype(np.float32))
    W3C = np.zeros((G, C * G), np.float32)
    hi = csum.astype(np.float32).astype(bf16).astype(np.float32)
    lo = (csum - hi.astype(np.float64)).astype(np.float32)
    for g in range(G):
        for c in range(C):
            W3C[g, G * c + g] = hi[c]
            W3C[(g + 1) % G, G * c + g] = lo[c]
    ONES = np.ones((G, NT), np.float32)
    return (W1, W2, W3bf, W3C.astype(bf16), ONES.astype(bf16))


def _stage_inputs(rgb_mask: np.ndarray, colors: np.ndarray):
    W1, W2, W3, W3C, ONES = _weights(np.asarray(colors, np.float32))
    in_maps = []
    for i in range(NCORES):
        xc = np.asarray(rgb_mask[BPC * i:BPC * (i + 1)], np.float32)
        xc = np.transpose(xc, (1, 0, 2, 3)).reshape(C * G, REG)
        in_maps.append({
            "x": np.ascontiguousarray(xc),
            "w1": W1, "w2": W2, "w3": W3, "w3c": W3C, "ones": ONES,
        })
    return in_maps


def _gather_outputs(results):
    outs = []
    for i in range(NCORES):
        yb = results[i]["y"].reshape(C, BPC, H, W)
        outs.append(np.transpose(yb, (1, 0, 2, 3)))
    return np.ascontiguousarray(np.concatenate(outs, axis=0))


def run(rgb_mask, colors, trace=False, reps=1, **kw):
    from concourse.bass_utils import run_bass_kernel_spmd

    nc = _build_nc(reps)
    in_maps = _stage_inputs(rgb_mask, colors)
    res = run_bass_kernel_spmd(nc, in_maps, core_ids=list(range(NCORES)),
                               trace=trace, **kw)
    return _gather_outputs(res.results), res


def kernel(rgb_mask, colors):
    out, _ = run(rgb_mask, colors)
    return out
